# revision 1
# baseline (speedup 1.0000x reference)
# FVSBN kernel for Trainium2: out = x @ (W * tril(-1)).T + b
#   x: [65536, 764] f32, W: [764, 764] f32, b: [764] f32 -> out: [65536, 764] f32
#
# Strategy: data-parallel over batch across 8 NeuronCores (8192 rows each).
# On each core we compute out^T = Wm^T-tiles.T @ x^T as a block-lower-triangular
# matmul: the strictly-lower-triangular mask means output tile row nt only needs
# contraction tiles dt <= nt (21 of 36 tile pairs).
#   - stationary operand (lhsT): Wm^T tile [128 d, 128 n]  (host packs the 21
#     used tiles contiguously)
#   - moving operand (rhs): x^T tile [128 d, 512 b]  (host pre-transposes x so
#     device DMAs are contiguous)
#   - psum [128 n, 512 b] accumulates over dt; eviction fuses the bias add
#     (per-partition scalar) on the vector engine.
# Host gathers by transposing each core's out^T back.

import numpy as np

B = 65536
D = 764
NCORES = 8
BPC = B // NCORES  # 8192 rows per core
P = 128
NT = 6  # ceil(764/128)
DP = NT * P  # 768, zero-padded depth
BB = 512  # matmul moving free dim == psum bank width (fp32)
PAIRS = [(nt, dt) for nt in range(NT) for dt in range(nt + 1)]
PAIR_IDX = {p: j for j, p in enumerate(PAIRS)}
NPAIR = len(PAIRS)  # 21

# device compute dtypes (np side); "float32r" = full-rate fp32 matmul mode
X_DT = "float16"  # dtype of x / W on device
OUT_DT = "float16"  # dtype out^T is written in
MM_DT = "float16"  # dtype the PE sees for the matmul operands


def _np_dt(name):
    import ml_dtypes

    return {
        "float32": np.float32,
        "float16": np.float16,
        "bfloat16": ml_dtypes.bfloat16,
    }[name]


def _build(
    bpc,
    x_dt_str=X_DT,
    out_dt_str=OUT_DT,
    mm_dt_str=MM_DT,
    reps=1,
    ablate_x=False,
    ablate_out=False,
    xc=2048,
):
    import concourse.mybir as mybir
    from concourse import bacc
    from concourse.tile import TileContext

    x_dt = getattr(mybir.dt, x_dt_str)
    out_dt = getattr(mybir.dt, out_dt_str)
    mm_dt = getattr(mybir.dt, mm_dt_str)
    f32 = mybir.dt.float32
    nbb = bpc // BB

    nc = bacc.Bacc("TRN2", target_bir_lowering=False, debug=False)
    xT = nc.dram_tensor("xt", [DP, bpc], x_dt, kind="ExternalInput")
    wt = nc.dram_tensor("wt", [P, NPAIR * P], x_dt, kind="ExternalInput")
    bias = nc.dram_tensor("bias", [P, NT], f32, kind="ExternalInput")
    outT = nc.dram_tensor("outt", [DP, bpc], out_dt, kind="ExternalOutput")

    def mm(ap):
        return ap if ap.dtype == mm_dt else ap.bitcast(mm_dt)

    # b is split in halves for x-load/compute pipelining; within a half,
    # 512-wide matmul blocks. x^T stays fully resident in SBUF (96KB/part fp16).
    HB = bpc // 2
    nhb = HB // BB

    with TileContext(nc) as tc:
        with (
            tc.tile_pool(name="wpool", bufs=1) as wpool,
            tc.tile_pool(name="bpool", bufs=1) as bpool,
            tc.tile_pool(name="xpool", bufs=1) as xpool,
            tc.tile_pool(name="opool", bufs=3) as opool,
            tc.tile_pool(name="pspool", bufs=8, space="PSUM") as pspool,
        ):
            w_sb = wpool.tile([P, NPAIR * P], x_dt)
            nc.sync.dma_start(out=w_sb, in_=wt.ap())
            bias_sb = bpool.tile([P, NT], f32)
            nc.sync.dma_start(out=bias_sb, in_=bias.ap())

            xres = [
                xpool.tile([P, bpc], x_dt, tag=f"xres{t}", name=f"xres{t}")
                for t in range(NT)
            ]

            XC = min(xc, HB)  # x-load chunk width

            def load_x():
                for half in range(2):
                    for t in range(NT):
                        for c0 in range(half * HB, (half + 1) * HB, XC):
                            nc.sync.dma_start(
                                out=xres[t][:, c0 : c0 + XC],
                                in_=xT.ap()[t * P : (t + 1) * P, c0 : c0 + XC],
                            )

            def body():
                if not ablate_x:
                    load_x()
                GRP = min(4, nhb)  # bgs sharing one weight load burst
                for half in range(2):
                    for nt in range(NT):
                        o_nt = opool.tile([P, HB], out_dt)
                        for grp in range(nhb // GRP):
                            pss = [
                                pspool.tile([P, BB], f32, name="ps")
                                for _ in range(GRP)
                            ]
                            for dt_ in range(nt + 1):
                                j = PAIR_IDX[(nt, dt_)]
                                for g4 in range(GRP):
                                    c0 = half * HB + (grp * GRP + g4) * BB
                                    nc.tensor.matmul(
                                        pss[g4],
                                        mm(w_sb[:, j * P : (j + 1) * P]),
                                        mm(xres[dt_][:, c0 : c0 + BB]),
                                        start=(dt_ == 0),
                                        stop=(dt_ == nt),
                                    )
                            for g4 in range(GRP):
                                bg = grp * GRP + g4
                                nc.vector.tensor_add(
                                    out=o_nt[:, bg * BB : (bg + 1) * BB],
                                    in0=pss[g4],
                                    in1=bias_sb[:, nt : nt + 1].broadcast_to([P, BB]),
                                )
                        # out-writes ride the scalar-engine HWDGE ring so they
                        # don't queue behind x-loads on the sync ring
                        if ablate_out:
                            nc.scalar.dma_start(
                                out=outT.ap()[nt * P : (nt + 1) * P, 0:8],
                                in_=o_nt[:, 0:8],
                            )
                        else:
                            nc.scalar.dma_start(
                                out=outT.ap()[
                                    nt * P : (nt + 1) * P, half * HB : (half + 1) * HB
                                ],
                                in_=o_nt,
                            )

            if ablate_x:
                load_x()
            if reps == 1:
                body()
            else:
                with tc.For_i(0, reps, 1, hint_engines=(mybir.EngineType.PE,)):
                    body()
    nc.compile()
    _dedup_ldweights(nc, mybir)
    return nc


def _dedup_ldweights(nc, mybir):
    """Remove back-to-back redundant LDWEIGHTS: within a basic block, a
    Ldweights whose weight AP matches the previous PE weight load (with no
    intervening write to that SBUF region and no semaphores attached) leaves
    the PE array state unchanged and can be dropped."""
    n_removed = 0
    for blk in nc.m.functions[0].blocks:
        il = blk.instructions
        last_sig = None
        to_remove = []
        for inst in il:
            if isinstance(inst, mybir.InstLdweights):
                a = inst.ins[0]
                sig = (
                    a.memref,
                    a.offset,
                    str(a.ap),
                    str(a.dtype),
                    bool(inst.is_transpose),
                )
                if (
                    sig == last_sig
                    and not inst.has_wait()
                    and not inst.has_update()
                ):
                    to_remove.append(inst)
                else:
                    last_sig = sig
            elif isinstance(inst, mybir.InstMatmult):
                continue
            else:
                # any other instruction writing SBUF could touch the weight
                # region; only DMA writes can hit w_sb in this kernel, but be
                # conservative: reset on any instruction with outputs into the
                # same memref as the tracked weights.
                if last_sig is not None:
                    try:
                        outs = inst.outs
                    except AttributeError:
                        outs = []
                    for o in outs or []:
                        if getattr(o, "memref", None) == last_sig[0]:
                            last_sig = None
                            break
        for inst in to_remove:
            il.remove(inst)
        n_removed += len(to_remove)
    return n_removed


def _prep_shared(W, b, x_np_dt):
    # masked transposed weights, packed as the 21 lower-triangular 128x128 tiles
    Wm = W * np.tril(np.ones((D, D), np.float32), k=-1)
    WT = np.zeros((DP, DP), np.float32)
    WT[:D, :D] = Wm.T  # WT[d, n] = Wm[n, d]
    w_packed = np.empty((P, NPAIR, P), x_np_dt)
    for j, (nt, dt_) in enumerate(PAIRS):
        w_packed[:, j, :] = WT[dt_ * P : (dt_ + 1) * P, nt * P : (nt + 1) * P]
    w_packed = np.ascontiguousarray(w_packed.reshape(P, NPAIR * P))
    bias_pad = np.zeros(DP, np.float32)
    bias_pad[:D] = b
    bias_t = np.ascontiguousarray(bias_pad.reshape(NT, P).T)  # [p, t] = b[t*128+p]
    return w_packed, bias_t


def kernel(x, W, b):
    from concourse.bass_utils import run_bass_kernel_spmd

    x_np_dt = _np_dt(X_DT)
    out_np_dt = _np_dt(OUT_DT)
    nc = _build(BPC)
    w_packed, bias_t = _prep_shared(W, b, x_np_dt)

    in_maps = []
    for c in range(NCORES):
        xs = x[c * BPC : (c + 1) * BPC]
        xT = np.zeros((DP, BPC), x_np_dt)
        xT[:D] = xs.T
        in_maps.append({"xt": xT, "wt": w_packed, "bias": bias_t})

    res = run_bass_kernel_spmd(nc, in_maps, core_ids=list(range(NCORES)))

    out = np.empty((B, D), np.float32)
    for c in range(NCORES):
        out[c * BPC : (c + 1) * BPC] = (
            res.results[c]["outt"][:D].astype(np.float32).T
        )
    return out

